# revision 1
# baseline (speedup 1.0000x reference)
"""Deformable 3D conv net on 8 Trainium2 NeuronCores (Bass/Tile).

Sharding: core (b, q) = batch b in {0,1} x D-quarter q in {0..3};
each core computes out[b, :, 12q:12q+12, :, :] from a padded x slab.

Per-core algorithm (exact trilinear, 5-wide window, exact for |off|<=2;
offsets clamped to [-2,2] on device; actual max |off| ~ 1.83):
  1. PE off-conv: off[81, 48,48] per d-slice, contraction K=96
     (3 w-shift replicas x 32 channels) accumulated over 9 (kd,kh) taps.
  2. Per tap k: zeta[(dd,dh,dw), h, w] = hat(od-dd)hat(oh-dh)hat(ow-dw)
     built with doubling copies + tensor_scalar chains (bf16, 125 rows).
  3. Per channel c: xr = 125 delta-shifted replicas of padded x channel c
     (one replicating DMA); per tap: P = zeta * xr_window (DVE bf16);
     PE matmul K=125 with stationary w_dc[o,c,k] broadcast over rows
     accumulates out[32, h, w] in PSUM across all (k, c).
"""

import numpy as np
import ml_dtypes

import concourse.bass as bass
import concourse.bacc as bacc
import concourse.mybir as mybir
from concourse.tile import TileContext
from concourse.bass_utils import run_bass_kernel_spmd

B, C, O, S = 2, 32, 32, 48
KS, KV = 3, 27
PAD = 4
DP = 12                 # output D per core
DPP = DP + 2 * PAD      # 20
HP = WP = S + 2 * PAD   # 56
HWP = HP * WP           # 3136
NPAD = DPP * HWP        # 62720
NDELTA = 125

F32 = mybir.dt.float32
BF16 = mybir.dt.bfloat16
ALU = mybir.AluOpType
ACTF = mybir.ActivationFunctionType

TAP_GROUPS = [list(range(0, 14)), list(range(14, 27))]
HCHUNKS = [(0, 10), (10, 10), (20, 10), (30, 10), (40, 8)]  # h-row chunks
NS_LOOP = DP  # number of d-slices traced (reduce for simulation tests)
LAST_RESULTS = None


# ---------------------------------------------------------------- host prep
def _build_core_inputs(x, w_off, b_off, w_dc, b_dc, b, q):
    xp = np.zeros((C, DPP, HP, WP), np.float32)
    d0 = DP * q - PAD
    lo, hi = max(0, -d0), min(DPP, S - d0)
    xp[:, lo:hi, PAD:PAD + S, PAD:PAD + S] = x[b, :, d0 + lo:d0 + hi]

    # x3[32g+c, d, h, w] = xp[c, d, h, w + (g-1)]  (wrap lands in zero pad)
    x3 = np.zeros((96, DPP, HP, WP), np.float32)
    for g in range(3):
        x3[32 * g:32 * g + 32] = np.roll(xp, -(g - 1), axis=3)
    x3 = x3.reshape(96, NPAD).astype(ml_dtypes.bfloat16)

    x_bf = xp.reshape(C, NPAD).astype(ml_dtypes.bfloat16)

    # w_off9: [9*96, 81]: chunk (kd,kh), rows (kw, c), cols m = 3k + axis
    woff = w_off.reshape(KV, 3, C, KS, KS, KS)
    w_off9 = np.zeros((9, 96, 81), np.float32)
    for kd in range(3):
        for kh in range(3):
            ch = kd * 3 + kh
            for kw in range(3):
                blk = woff[:, :, :, kd, kh, kw]          # (k, ax, c)
                w_off9[ch, 32 * kw:32 * kw + 32, :] = \
                    blk.transpose(2, 0, 1).reshape(C, KV * 3)
    w_off9 = w_off9.astype(ml_dtypes.bfloat16)

    # wdc_rep: [128, KV*C*O]: rows = delta (125 used), free (k, c, o)
    wdcf = w_dc.reshape(O, C, KV)
    wdc = np.zeros((128, KV * C * O), np.float32)
    wdc[:NDELTA, :] = wdcf.transpose(2, 1, 0).reshape(KV * C * O)[None, :]
    wdc = wdc.astype(ml_dtypes.bfloat16)

    dd = np.repeat(np.arange(-2, 3), 25).astype(np.float32)[:, None]
    dh = np.tile(np.repeat(np.arange(-2, 3), 5), 5).astype(np.float32)[:, None]
    dw = np.tile(np.arange(-2, 3), 25).astype(np.float32)[:, None]

    return {
        "x3": np.ascontiguousarray(x3),
        "x_bf": np.ascontiguousarray(x_bf),
        "w_off9": np.ascontiguousarray(w_off9.transpose(1, 0, 2).reshape(96, 9 * 81)),
        "wdc_rep": np.ascontiguousarray(wdc),
        "b_off": np.ascontiguousarray(b_off.astype(np.float32).reshape(81, 1)),
        "b_dc": np.ascontiguousarray(b_dc.astype(np.float32).reshape(32, 1)),
        "dvec_d": dd, "dvec_h": dh, "dvec_w": dw,
    }


# ---------------------------------------------------------------- device IR
def _win_ap(dram_row_ap, offset, ap_dims):
    a = dram_row_ap.copy()
    a.ap = mybir.VecI64Pair(ap_dims)
    a.offset = offset
    return a


def _build_zeta(nc, pool, ds, k, off_dram, dvecs, zeta, hats, bcs):
    """hats[ax] = max(1 - |off_row - dvec|, 0) on 125 rows; zeta = prod."""
    for ax in range(3):
        bc = bcs[ax]
        src = _win_ap(off_dram[0:1, :],
                      (ds * 81 + 3 * k + ax) * S * S,
                      [(0, NDELTA), (1, S * S)])
        nc.sync.dma_start(bc.rearrange("p h w -> p (h w)"), src)
        # u = |dvec - bc| ; h = relu(1 - u)   (both on the scalar engine)
        nc.scalar.activation(bc[:], bc[:], ACTF.Abs,
                             bias=dvecs[ax][:, :], scale=-1.0)
        nc.scalar.activation(hats[ax][:], bc[:], ACTF.Relu,
                             bias=1.0, scale=-1.0)
    nc.vector.tensor_tensor(zeta[:], hats[0][:], hats[1][:], ALU.mult)
    nc.vector.tensor_tensor(zeta[:], zeta[:], hats[2][:], ALU.mult)


def build_kernel(nc: bass.Bass):
    x3_d = nc.dram_tensor("x3", [96, NPAD], BF16, kind="ExternalInput")
    xbf_d = nc.dram_tensor("x_bf", [C, NPAD], BF16, kind="ExternalInput")
    woff_d = nc.dram_tensor("w_off9", [96, 9 * 81], BF16, kind="ExternalInput")
    wdc_d = nc.dram_tensor("wdc_rep", [128, KV * C * O], BF16,
                           kind="ExternalInput")
    boff_d = nc.dram_tensor("b_off", [81, 1], F32, kind="ExternalInput")
    bdc_d = nc.dram_tensor("b_dc", [32, 1], F32, kind="ExternalInput")
    dv_d = [nc.dram_tensor(n, [NDELTA, 1], F32, kind="ExternalInput")
            for n in ("dvec_d", "dvec_h", "dvec_w")]
    off_dram = nc.dram_tensor("off_scratch", [1, NS_LOOP * 81 * S * S], F32,
                              kind="Internal")
    out_d = nc.dram_tensor("out", [O, NS_LOOP * S * S], F32, kind="ExternalOutput")

    with TileContext(nc) as tc:
        with tc.tile_pool(name="fixed", bufs=1) as fixed:
            woff_s = fixed.tile([96, 9 * 81], BF16)
            nc.sync.dma_start(woff_s[:, :], woff_d[:, :])
            wdc_s = fixed.tile([128, KV * C * O], BF16)
            nc.sync.dma_start(wdc_s[:, :], wdc_d[:, :])
            boff_s = fixed.tile([81, 1], F32)
            nc.sync.dma_start(boff_s[:, :], boff_d[:, :])
            bdc_s = fixed.tile([32, 1], F32)
            nc.sync.dma_start(bdc_s[:, :], bdc_d[:, :])
            dvecs = []
            for i, t in enumerate(dv_d):
                dv = fixed.tile([NDELTA, 1], F32, name=f"dv{i}")
                nc.sync.dma_start(dv[:, :], t[:, :])
                dvecs.append(dv)

            # warm fixed tiles on DVE once so later DVE instructions don't
            # each carry a DMA-sem wait (HW wait-slot limit)
            warm = fixed.tile([1, 8], F32)
            for wsrc in [boff_s, bdc_s] + dvecs:
                nc.vector.tensor_copy(warm[0:1, 0:1], wsrc[0:1, 0:1])

            for ds in range(NS_LOOP):
                _do_slice(nc, tc, ds, x3_d, xbf_d, out_d, off_dram,
                          woff_s, wdc_s, boff_s, bdc_s, dvecs)
    return nc


def _do_slice(nc, tc, ds, x3_d, xbf_d, out_d, off_dram,
              woff_s, wdc_s, boff_s, bdc_s, dvecs):
    dpad = ds + PAD
    with tc.tile_pool(name=f"sl{ds}", bufs=1) as pool, \
         tc.tile_pool(name=f"psum{ds}", bufs=1, space="PSUM") as psp:

        # ---------------- off-conv ----------------
        x3s = pool.tile([96, 3, HP, WP], BF16, name=f"x3s{ds}", tag="x3s")
        nc.sync.dma_start(
            x3s.rearrange("p a h w -> p (a h w)"),
            x3_d[:, (dpad - 1) * HWP:(dpad + 2) * HWP])
        off = pool.tile([81, S, S], F32, name=f"off{ds}", tag="off")
        for hc, (hb, hn) in enumerate(HCHUNKS):
            ps = psp.tile([81, hn, S], F32, name=f"offps{ds}_{hc}",
                          tag="offps")
            for i in range(9):
                kd, kh = i // 3, i % 3
                rhs = x3s[:, kd, 3 + kh + hb:3 + kh + hb + hn, 4:52]
                nc.tensor.matmul(ps[:], woff_s[:, i * 81:(i + 1) * 81],
                                 rhs, start=(i == 0), stop=(i == 8))
            # evict + bias + clamp to [-2, 2]
            nc.vector.tensor_scalar(off[:, hb:hb + hn, :], ps[:],
                                    boff_s[:, :], 2.0, ALU.add, ALU.min)
        nc.vector.tensor_scalar(off[:], off[:], -2.0, None, ALU.max)
        nc.sync.dma_start(
            _win_ap(off_dram[0:1, :], ds * 81 * S * S,
                    [(S * S, 81), (1, S * S)]),
            off.rearrange("p h w -> p (h w)"))

        # ---------------- accumulators ----------------
        accs = [psp.tile([O, hn, S], F32, name=f"acc{ds}_{ci}", tag=f"acc{ci}")
                for ci, (hb, hn) in enumerate(HCHUNKS)]

        bc0 = pool.tile([NDELTA, S, S], F32, name=f"bc{ds}", tag="bc",
                        bufs=1)
        bcs = [bc0, bc0, bc0]
        hats = [pool.tile([NDELTA, S, S], BF16, name=f"hat{ds}_{ax}",
                          tag=f"hat{ax}") for ax in range(3)]
        first_mm = [True] * len(HCHUNKS)
        for gi, taps in enumerate(TAP_GROUPS):
            zetas = {}
            for k in taps:
                z = pool.tile([NDELTA, S, S], BF16, name=f"z{ds}_{k}",
                              tag=f"z{k % 14}")
                _build_zeta(nc, pool, ds, k, off_dram, dvecs, z, hats, bcs)
                zetas[k] = z
            last = (gi == len(TAP_GROUPS) - 1)
            for c in range(C):
                xr = pool.tile([NDELTA, 3, HP, WP], BF16,
                               name=f"xr{ds}_{gi}_{c}", tag="xr", bufs=1)
                xrf = xr.rearrange("p a h w -> p (a h w)")
                for a5 in range(5):
                    src = _win_ap(
                        xbf_d[c:c + 1, :],
                        c * NPAD + (dpad - 3 + a5) * HWP - 2 * WP - 2,
                        [(WP, 5), (1, 5), (1, 3 * HWP)])
                    nc.sync.dma_start(xrf[25 * a5:25 * a5 + 25, :], src)
                for k in taps:
                    kd, kh, kw = k // 9, (k // 3) % 3, k % 3
                    win = xr[:, kd, 3 + kh:3 + kh + S, 3 + kw:3 + kw + S]
                    p = pool.tile([NDELTA, S, S], BF16,
                                  name=f"p{ds}_{gi}_{c}_{k}", tag="ptile",
                                  bufs=2)
                    nc.vector.tensor_tensor(p[:], zetas[k][:], win, ALU.mult)
                    wsl = wdc_s[0:NDELTA, (k * C + c) * O:(k * C + c + 1) * O]
                    fin = last and (c == C - 1) and (k == taps[-1])
                    for ci, (hb, hn) in enumerate(HCHUNKS):
                        nc.tensor.matmul(accs[ci][:], wsl,
                                         p[:, hb:hb + hn, :],
                                         start=first_mm[ci], stop=fin)
                        first_mm[ci] = False

        # ---------------- evict ----------------
        outp = pool.tile([O, S, S], F32, name=f"outp{ds}", tag="outp")
        for ci, (hb, hn) in enumerate(HCHUNKS):
            nc.vector.tensor_scalar(outp[:, hb:hb + hn, :], accs[ci][:],
                                    bdc_s[:, :], None, ALU.add)
        nc.sync.dma_start(out_d[:, ds * S * S:(ds + 1) * S * S],
                          outp.rearrange("p h w -> p (h w)"))


# ---------------------------------------------------------------- entry
def kernel(x, w_off, b_off, w_dc, b_dc):
    x = np.asarray(x, np.float32)
    w_off = np.asarray(w_off, np.float32)
    b_off = np.asarray(b_off, np.float32)
    w_dc = np.asarray(w_dc, np.float32)
    b_dc = np.asarray(b_dc, np.float32)

    in_maps = [_build_core_inputs(x, w_off, b_off, w_dc, b_dc,
                                  core // 4, core % 4) for core in range(8)]

    nc = bacc.Bacc("TRN2", target_bir_lowering=False, debug=False,
                   enable_asserts=False, num_devices=8)
    build_kernel(nc)
    if not nc.is_finalized():
        nc.finalize()

    global LAST_RESULTS
    LAST_RESULTS = run_bass_kernel_spmd(nc, in_maps, list(range(8)))
    res = LAST_RESULTS.results

    out = np.zeros((B, O, S, S, S), np.float32)
    for core in range(8):
        b, q = core // 4, core % 4
        out[b, :, DP * q:DP * q + NS_LOOP] = \
            res[core]["out"].reshape(O, NS_LOOP, S, S).astype(np.float32)
    return out



# revision 3
# speedup vs baseline: 29.4529x; 29.4529x over previous
"""Deformable 3D conv net on 8 Trainium2 NeuronCores (Bass/Tile).

Sharding: core (b, q) = batch b in {0,1} x D-quarter q in {0..3};
each core computes out[b, :, 12q:12q+12, :, :] from a padded x slab.

Per-core algorithm (exact trilinear, 5-wide window, exact for |off|<=2;
offsets clamped to [-2,2] on device; actual max |off| ~ 1.83):
  1. PE off-conv: off[81, 48,48] per d-slice, contraction K=96
     (3 w-shift replicas x 32 channels) accumulated over 9 (kd,kh) taps.
  2. Per tap k: hat5[ax][j,n] = relu(1-|off-j|) built on [5, 2304] tiles
     (scalar engine), replicated to 125 rows via stride-0 DMA reads and
     combined into zeta[(dd,dh,dw), n] with two DVE multiplies.
  3. Taps grouped by kd (3 groups of 9). Per (group, channel): xr = 125
     delta-shifted replicas of one padded x d-plane (one replicating
     DMA, [125, 3136] bf16); per tap: P = zeta * xr_window (DVE bf16);
     PE matmul K=125 with stationary w_dc[o,c,k] broadcast over rows
     accumulates out[32, h, w] in PSUM across all (g, c, k).
"""

import numpy as np
import ml_dtypes

import concourse.bass as bass
import concourse.bacc as bacc
import concourse.mybir as mybir
from concourse.tile import TileContext
from concourse.bass_utils import run_bass_kernel_spmd

B, C, O, S = 2, 32, 32, 48
KS, KV = 3, 27
PAD = 4
DP = 12                 # output D per core
DPP = DP + 2 * PAD      # 20
HP = WP = S + 2 * PAD   # 56
HWP = HP * WP           # 3136
NPAD = DPP * HWP        # 62720
NDELTA = 125
SS = S * S              # 2304

F32 = mybir.dt.float32
BF16 = mybir.dt.bfloat16
ALU = mybir.AluOpType
ACTF = mybir.ActivationFunctionType

HCHUNKS = [(0, 10), (10, 10), (20, 10), (30, 10), (40, 8)]  # h-row chunks
NS_LOOP = DP  # number of d-slices traced (reduce for simulation tests)
LAST_RESULTS = None


# ---------------------------------------------------------------- host prep
def _build_core_inputs(x, w_off, b_off, w_dc, b_dc, b, q):
    xp = np.zeros((C, DPP, HP, WP), np.float32)
    d0 = DP * q - PAD
    lo, hi = max(0, -d0), min(DPP, S - d0)
    xp[:, lo:hi, PAD:PAD + S, PAD:PAD + S] = x[b, :, d0 + lo:d0 + hi]

    # x3[32g+c, d, h, w] = xp[c, d, h, w + (g-1)]  (wrap lands in zero pad)
    x3 = np.zeros((96, DPP, HP, WP), np.float32)
    for g in range(3):
        x3[32 * g:32 * g + 32] = np.roll(xp, -(g - 1), axis=3)
    x3 = x3.reshape(96, NPAD).astype(ml_dtypes.bfloat16)

    x_bf = xp.reshape(C, NPAD).astype(ml_dtypes.bfloat16)

    # w_off9: [96, 9*81]: chunk (kd,kh), rows (kw, c), cols m = 3k + axis
    woff = w_off.reshape(KV, 3, C, KS, KS, KS)
    w_off9 = np.zeros((9, 96, 81), np.float32)
    for kd in range(3):
        for kh in range(3):
            ch = kd * 3 + kh
            for kw in range(3):
                blk = woff[:, :, :, kd, kh, kw]          # (k, ax, c)
                w_off9[ch, 32 * kw:32 * kw + 32, :] = \
                    blk.transpose(2, 0, 1).reshape(C, KV * 3)
    w_off9 = w_off9.astype(ml_dtypes.bfloat16)

    # wdc_g: [128, (g, c, kl, o)]: rows = delta (125 used), per-(group,c)
    # stationary slabs of 9 local taps x 32 outputs
    wdcf = w_dc.reshape(O, C, KV)            # k = 9*kd + 3*kh + kw
    wdc = np.zeros((128, 3, C, 9 * O), np.float32)
    for g in range(3):
        for kl in range(9):
            k = 9 * g + kl
            # [c, o] block
            wdc[:NDELTA, g, :, kl * O:(kl + 1) * O] = \
                wdcf[:, :, k].T[None, :, :]
    wdc = wdc.reshape(128, 3 * C * 9 * O).astype(ml_dtypes.bfloat16)

    dvec5 = np.arange(-2, 3).astype(np.float32).reshape(5, 1)

    return {
        "x3": np.ascontiguousarray(x3),
        "x_bf": np.ascontiguousarray(x_bf),
        "w_off9": np.ascontiguousarray(w_off9.transpose(1, 0, 2).reshape(96, 9 * 81)),
        "wdc_g": np.ascontiguousarray(wdc),
        "b_off": np.ascontiguousarray(b_off.astype(np.float32).reshape(81, 1)),
        "b_dc": np.ascontiguousarray(b_dc.astype(np.float32).reshape(32, 1)),
        "dvec5": dvec5,
    }


# ---------------------------------------------------------------- device IR
def _win_ap(dram_row_ap, offset, ap_dims):
    a = dram_row_ap.copy()
    a.ap = mybir.VecI64Pair(ap_dims)
    a.offset = offset
    return a


def build_kernel(nc: bass.Bass):
    x3_d = nc.dram_tensor("x3", [96, NPAD], BF16, kind="ExternalInput")
    xbf_d = nc.dram_tensor("x_bf", [C, NPAD], BF16, kind="ExternalInput")
    woff_d = nc.dram_tensor("w_off9", [96, 9 * 81], BF16, kind="ExternalInput")
    wdc_d = nc.dram_tensor("wdc_g", [128, 3 * C * 9 * O], BF16,
                           kind="ExternalInput")
    boff_d = nc.dram_tensor("b_off", [81, 1], F32, kind="ExternalInput")
    bdc_d = nc.dram_tensor("b_dc", [32, 1], F32, kind="ExternalInput")
    dv_d = nc.dram_tensor("dvec5", [5, 1], F32, kind="ExternalInput")
    # scratch: bf16 offsets, per-tap hats and 2-axis zeta (DRAM-bounced
    # so stride-0 partition-replicating reads have a proven source)
    offbf_d = nc.dram_tensor("offbf", [1, NS_LOOP * 81 * SS], BF16,
                             kind="Internal")
    hat_d = nc.dram_tensor("hat", [1, NS_LOOP * KV * 3 * 5 * SS], BF16,
                           kind="Internal")
    z2_d = nc.dram_tensor("z2", [1, NS_LOOP * KV * 25 * SS], BF16,
                          kind="Internal")
    out_d = nc.dram_tensor("out", [O, NS_LOOP * SS], F32, kind="ExternalOutput")

    with TileContext(nc) as tc:
        with tc.tile_pool(name="fixed", bufs=1) as fixed, \
             tc.tile_pool(name="work", bufs=1) as work, \
             tc.tile_pool(name="psum", bufs=1, space="PSUM") as psp:
            woff_s = fixed.tile([96, 9 * 81], BF16)
            nc.sync.dma_start(woff_s[:, :], woff_d[:, :])
            boff_s = fixed.tile([81, 1], F32)
            nc.sync.dma_start(boff_s[:, :], boff_d[:, :])
            bdc_s = fixed.tile([32, 1], F32)
            nc.sync.dma_start(bdc_s[:, :], bdc_d[:, :])
            dv_s = fixed.tile([5, 1], F32)
            nc.sync.dma_start(dv_s[:, :], dv_d[:, :])

            # warm fixed tiles on DVE once so later DVE instructions don't
            # each carry a DMA-sem wait (HW wait-slot limit)
            warm = fixed.tile([1, 8], F32)
            for wsrc in [boff_s, bdc_s, dv_s]:
                nc.vector.tensor_copy(warm[0:1, 0:1], wsrc[0:1, 0:1])

            for ds in range(NS_LOOP):
                _do_slice(nc, tc, ds, x3_d, xbf_d, wdc_d, out_d,
                          offbf_d, hat_d, z2_d, work, psp,
                          woff_s, boff_s, bdc_s, dv_s)
    return nc


def _build_zeta(nc, work, ds, k, offbf_d, hat_d, z2_d, dv_s, zeta):
    """zeta[(dd,dh,dw), n] = prod_ax relu(1 - |off_ax - delta_ax|), bf16."""
    hat_base = ((ds * KV + k) * 3) * 5 * SS
    for ax in range(3):
        bc5 = work.tile([5, SS], BF16, name=f"bc5_{ds}_{k}_{ax}", tag="bc5",
                        bufs=1)
        src = _win_ap(offbf_d[0:1, :], (ds * 81 + 3 * k + ax) * SS,
                      [(0, 5), (1, SS)])
        nc.scalar.dma_start(bc5[:, :], src)
        # u = |dvec - od| ; hat = relu(1 - u)
        nc.scalar.activation(bc5[:], bc5[:], ACTF.Abs,
                             bias=dv_s[:, :], scale=-1.0)
        hat5 = work.tile([5, SS], BF16, name=f"hat5_{ds}_{k}_{ax}",
                         tag=f"hat5_{ax}", bufs=1)
        nc.scalar.activation(hat5[:], bc5[:], ACTF.Relu,
                             bias=1.0, scale=-1.0)
        nc.scalar.dma_start(
            _win_ap(hat_d[0:1, :], hat_base + ax * 5 * SS, [(SS, 5), (1, SS)]),
            hat5[:, :])
    # z2[(dh,dw), n] = hh[dh,n] * hw[dw,n]  on 25 rows
    z25a = work.tile([25, SS], BF16, name=f"z25a_{ds}_{k}", tag="z25a",
                     bufs=1)
    nc.scalar.dma_start(
        z25a[:, :],
        _win_ap(hat_d[0:1, :], hat_base + 1 * 5 * SS, [(SS, 5), (0, 5), (1, SS)]))
    z25b = work.tile([25, SS], BF16, name=f"z25b_{ds}_{k}", tag="z25b",
                     bufs=1)
    nc.scalar.dma_start(
        z25b[:, :],
        _win_ap(hat_d[0:1, :], hat_base + 2 * 5 * SS, [(0, 5), (SS, 5), (1, SS)]))
    z2 = work.tile([25, SS], BF16, name=f"z2_{ds}_{k}", tag="z2", bufs=1)
    nc.vector.tensor_tensor(z2[:], z25a[:], z25b[:], ALU.mult)
    z2_base = (ds * KV + k) * 25 * SS
    nc.scalar.dma_start(
        _win_ap(z2_d[0:1, :], z2_base, [(SS, 25), (1, SS)]), z2[:, :])
    # zeta = hd-rep (25x per dd) * z2-rep (tiled 5x)
    zr1 = work.tile([NDELTA, SS], BF16, name=f"zr1_{ds}_{k}", tag="zr1",
                    bufs=1)
    nc.sync.dma_start(
        zr1[:, :],
        _win_ap(hat_d[0:1, :], hat_base, [(SS, 5), (0, 25), (1, SS)]))
    zr2 = work.tile([NDELTA, SS], BF16, name=f"zr2_{ds}_{k}", tag="zr2",
                    bufs=1)
    nc.sync.dma_start(
        zr2[:, :],
        _win_ap(z2_d[0:1, :], z2_base, [(0, 5), (SS, 25), (1, SS)]))
    nc.vector.tensor_tensor(zeta[:], zr1[:], zr2[:], ALU.mult)


def _do_slice(nc, tc, ds, x3_d, xbf_d, wdc_d, out_d, offbf_d, hat_d, z2_d,
              work, psp, woff_s, boff_s, bdc_s, dv_s):
    dpad = ds + PAD

    # ---------------- off-conv ----------------
    x3s = work.tile([96, 3, HP, WP], BF16, name=f"x3s{ds}", tag="x3s")
    nc.sync.dma_start(
        x3s.rearrange("p a h w -> p (a h w)"),
        x3_d[:, (dpad - 1) * HWP:(dpad + 2) * HWP])
    off = work.tile([81, S, S], F32, name=f"off{ds}", tag="off")
    for hc, (hb, hn) in enumerate(HCHUNKS):
        ps = psp.tile([81, hn, S], F32, name=f"offps{ds}_{hc}", tag="offps")
        for i in range(9):
            kd, kh = i // 3, i % 3
            rhs = x3s[:, kd, 3 + kh + hb:3 + kh + hb + hn, 4:52]
            nc.tensor.matmul(ps[:], woff_s[:, i * 81:(i + 1) * 81],
                             rhs, start=(i == 0), stop=(i == 8))
        # evict + bias + clamp to [-2, 2]
        nc.vector.tensor_scalar(off[:, hb:hb + hn, :], ps[:],
                                boff_s[:, :], 2.0, ALU.add, ALU.min)
    nc.vector.tensor_scalar(off[:], off[:], -2.0, None, ALU.max)
    off_bf = work.tile([81, S, S], BF16, name=f"offbf{ds}", tag="offb")
    nc.vector.tensor_copy(off_bf[:], off[:])
    nc.sync.dma_start(
        _win_ap(offbf_d[0:1, :], ds * 81 * SS, [(SS, 81), (1, SS)]),
        off_bf.rearrange("p h w -> p (h w)"))

    # ---------------- accumulators ----------------
    accs = [psp.tile([O, hn, S], F32, name=f"acc{ds}_{ci}", tag=f"acc{ci}")
            for ci, (hb, hn) in enumerate(HCHUNKS)]

    # ---------------- 3 kd-groups of 9 taps ----------------
    for g in range(3):
        zetas = {}
        for kl in range(9):
            k = 9 * g + kl
            z = work.tile([NDELTA, SS], BF16, name=f"z{ds}_{k}",
                          tag=f"zeta{g % 2}_{kl}")
            _build_zeta(nc, work, ds, k, offbf_d, hat_d, z2_d, dv_s, z)
            zetas[k] = z
        last_g = (g == 2)
        for c in range(C):
            xr = work.tile([NDELTA, HP, WP], BF16, name=f"xr{ds}_{g}_{c}",
                           tag="xr", bufs=2)
            xrf = xr.rearrange("p h w -> p (h w)")
            dma_eng = nc.sync if (c % 2 == 0) else nc.scalar
            for a5 in range(5):
                src = _win_ap(
                    xbf_d[c:c + 1, :],
                    c * NPAD + (dpad - 3 + a5 + g) * HWP - 2 * WP - 2,
                    [(WP, 5), (1, 5), (1, HWP)])
                dma_eng.dma_start(xrf[25 * a5:25 * a5 + 25, :], src)
            wgc = work.tile([NDELTA, 9 * O], BF16, name=f"w{ds}_{g}_{c}",
                            tag="wgc", bufs=2)
            nc.sync.dma_start(wgc[:, :],
                              wdc_d[0:NDELTA,
                                    (g * C + c) * 9 * O:(g * C + c + 1) * 9 * O])
            for kl in range(9):
                k = 9 * g + kl
                kh, kw = kl // 3, kl % 3
                win = xr[:, 3 + kh:3 + kh + S, 3 + kw:3 + kw + S]
                p = work.tile([NDELTA, S, S], BF16, name=f"p{ds}_{g}_{c}_{kl}",
                              tag="ptile", bufs=3)
                nc.vector.tensor_tensor(p[:], zetas[k][:], win, ALU.mult)
                wsl = wgc[:, kl * O:(kl + 1) * O]
                fin = last_g and (c == C - 1) and (kl == 8)
                for ci, (hb, hn) in enumerate(HCHUNKS):
                    nc.tensor.matmul(accs[ci][:], wsl, p[:, hb:hb + hn, :],
                                     start=(g == 0 and c == 0 and kl == 0),
                                     stop=fin)

    # ---------------- evict ----------------
    outp = work.tile([O, S, S], F32, name=f"outp{ds}", tag="outp")
    for ci, (hb, hn) in enumerate(HCHUNKS):
        nc.vector.tensor_scalar(outp[:, hb:hb + hn, :], accs[ci][:],
                                bdc_s[:, :], None, ALU.add)
    nc.sync.dma_start(out_d[:, ds * SS:(ds + 1) * SS],
                      outp.rearrange("p h w -> p (h w)"))


# ---------------------------------------------------------------- entry
def kernel(x, w_off, b_off, w_dc, b_dc):
    x = np.asarray(x, np.float32)
    w_off = np.asarray(w_off, np.float32)
    b_off = np.asarray(b_off, np.float32)
    w_dc = np.asarray(w_dc, np.float32)
    b_dc = np.asarray(b_dc, np.float32)

    in_maps = [_build_core_inputs(x, w_off, b_off, w_dc, b_dc,
                                  core // 4, core % 4) for core in range(8)]

    nc = bacc.Bacc("TRN2", target_bir_lowering=False, debug=False,
                   enable_asserts=False, num_devices=8)
    build_kernel(nc)
    if not nc.is_finalized():
        nc.finalize()

    global LAST_RESULTS
    LAST_RESULTS = run_bass_kernel_spmd(nc, in_maps, list(range(8)))
    res = LAST_RESULTS.results

    out = np.zeros((B, O, S, S, S), np.float32)
    for core in range(8):
        b, q = core // 4, core % 4
        out[b, :, DP * q:DP * q + NS_LOOP] = \
            res[core]["out"].reshape(O, NS_LOOP, S, S).astype(np.float32)
    return out
